# revision 46
# baseline (speedup 1.0000x reference)
"""Trainium2 Bass kernel for nn_FC_STGNN_SSC (STGNN over conv sleep-features).

Data-parallel over the batch: each of 8 NeuronCores processes 4 of the 32
batch elements (96 of the 768 flattened conv rows). All weights replicated.

Per-core pipeline (PE matmuls in fp32r; map2/fc1 streams in bf16):
  conv stack as a 4-stage software pipeline (conv1 of pair i, conv2 of pair
  i-1, conv3-even of pair i-2, conv3-odd of pair i-3) so every cross-engine
  hand-off has a full ~3.5us tick of slack; epilogues are engine-balanced
  (ACT does e1/e2 BN+ReLU, DVE pools conv2 + conv3 and runs 1-in-6 conv2
  rows + conv3-odd rows as scale/bias with relu folded into the pooling max).
  map2 24448->256: bf16 W' streamed through a 12-deep SBUF ring (PE-bound);
    a3 kept bf16; BN+posenc applied transposed after the PE transpose.
  MPNN x2: batch-padded row layout (32 rows/batch quad); F^T built directly
    as gw^T @ AF^T matmuls with per-partition bias; -1e8*eye pre-filled into
    the Gram PSUM by one const matmul; +eye and the adjacency scatter done
    by tiny PE matmuls against constant 0/1 selectors into a banded B;
    message passing = 4 K=32 matmuls per block at tile position (0,0);
    lrelu chains fused into single Lrelu activations.
  FC head fully transposed (h^T [c,4] tiles): no PSUM->SBUF round trips,
    biases folded into per-partition ACT; fc1 weights prefetched after map2;
    output transposed on the way out by a custom-AP DMA.
"""

import dataclasses
import os
import sys
from contextlib import ExitStack

import numpy as np

if not any(os.path.isdir(os.path.join(p, "concourse")) for p in sys.path if p):
    sys.path.insert(0, "/opt/trn_rl_repo")

import ml_dtypes  # noqa: E402

import concourse.bass as bass  # noqa: E402,F401
import concourse.bacc as bacc  # noqa: E402
import concourse.mybir as mybir  # noqa: E402
import concourse.tile as tile  # noqa: E402

F32 = mybir.dt.float32
F32R = mybir.dt.float32r
BF16 = mybir.dt.bfloat16
ACTF = mybir.ActivationFunctionType
AL = mybir.AluOpType

# model dims
BS, TLEN, NNODE, DIM = 32, 6, 4, 1500
HID = 128
D2 = 256
LSTMH, LSTMO, KCONV = 64, 128, 3
CONV_OUT = 191
DECAY = 0.7
FEAT_IN = LSTMO * CONV_OUT  # 24448

NCORES = 8
BSH = BS // NCORES          # 4 batch elems per core
R = BSH * TLEN * NNODE      # 96 conv rows per core
PAIRS = R // 2              # 48
PGRP = 4                    # conv1 pairs per T1 tile

L1, P1 = 1500, 751
L2, P2 = 753, 377
L3, P3 = 381, 191

NW1, NW2 = 5, 3             # windows per batch elem (stride 1 / 2)
NG1, NG2 = BSH * NW1, BSH * NW2   # 20, 12 graphs
NGT = NG1 + NG2             # 32 graphs over both MPNN blocks

# const-vector column indices within the cv section
(C_S1, C_B1, C_S2, C_B2, C_S3, C_B3,
 C_SA1_0, C_SA1_1, C_BA1_0, C_BA1_1,
 C_SA2_0, C_SA2_1, C_BA2_0, C_BA2_1,
 C_SM1, C_BM1, C_SM2, C_BM2,
 C_FB1_0, C_FB1_1, C_FB2_0, C_FB2_1, C_FB3,
 C_SMAP_0, C_SMAP_1, C_GB1_0, C_GB1_1, C_GB2_0, C_GB2_1, C_FB4,
 NCV) = range(31)


# ---- const blob layouts: name -> (col offset, ncols)
def _mk_layout(sections):
    off, lay = 0, {}
    for name, w in sections:
        lay[name] = (off, w)
        off += w
    return lay, off


# conv weights first so the first (small) blob DMA unblocks conv1 quickly
BR_LAY, BR_COLS = _mk_layout([
    ("b1blk", 128), ("w2t", 384), ("w3t", 384),
    ("g1w", 512), ("g2w", 512), ("th1w", 256), ("th2w", 256),
    ("fc2w", 512), ("fc3w", 256), ("fc4w", 8),
    ("sel1", 32 * NW1), ("sel2", 32 * NW2),
    ("eyen8", 8), ("eyesel1", 32 * NW1), ("eyesel2", 32 * NW2),
    ("ident8", 8), ("identt", 8 * NGT), ("gsel", 8 * NGT),
])
BR_CONV = 896  # cols holding conv weights (first DMAs)
BF_LAY, BF_COLS = _mk_layout([
    ("cv", NCV), ("ident", 128), ("mask8", 8 * NGT),
    ("fb4t", BSH), ("btotT", D2),
])
BF_CONV = NCV  # cols needed by the conv stack (first bfsb DMA)

W_TILE = 8          # wprime l-chunks per DMA
N_WTILES = (P3 + W_TILE - 1) // W_TILE   # 24
N_WRING = 12
FC_CHUNKS = 8       # fc1 512-row chunks


def r32(x):
    return np.ascontiguousarray(x, dtype=np.float32)


def bf16(x):
    return np.asarray(x, np.float32).astype(ml_dtypes.bfloat16)


def round_fp32r(x):
    """fp32 -> fp32r (11-bit mantissa, RNE); matches walrus fp32_to_fp32r."""
    u = np.ascontiguousarray(x, np.float32).view(np.uint32).astype(np.uint64)
    r = ((u + 0x7FF + ((u >> 12) & 1)) & 0xFFFFF000).astype(np.uint32)
    return r.view(np.float32).reshape(np.shape(x))


def fold_bn(p, extra_bias=None, post_scale=1.0):
    """y = x*scale + bias  ==  post_scale * BN(x + extra_bias)."""
    g, be, m, v = (np.asarray(p[i], np.float64) for i in range(4))
    sc = g / np.sqrt(v + 1e-5)
    bi = be - m * sc
    if extra_bias is not None:
        bi = bi + np.asarray(extra_bias, np.float64) * sc
    return r32(sc * post_scale), r32(bi * post_scale)


def pos_encoding():
    pos = np.arange(TLEN, dtype=np.float32)[:, None]
    div = np.exp(np.arange(0, D2, 2, dtype=np.float32)
                 * (np.float32(-np.log(np.float32(100.0))) / np.float32(D2)))
    pe = np.zeros((TLEN, D2), np.float32)
    pe[:, 0::2] = np.sin(pos * div)
    pe[:, 1::2] = np.cos(pos * div)
    return pe


def prep_consts(inp):
    """Host-side constant prep (shared by all cores)."""
    br = np.zeros((128, BR_COLS), np.float32)

    def brput(name, rows, arr):
        c0, w = BR_LAY[name]
        a = np.asarray(arr, np.float32)
        assert a.shape == (rows, w), (name, a.shape)
        br[0:rows, c0:c0 + w] = a

    w1 = np.asarray(inp["conv1_w"], np.float32)
    b1blk = np.zeros((128, 128), np.float32)
    for ri in range(2):
        for t in range(KCONV):
            b1blk[3 * ri + t, 64 * ri:64 * ri + 64] = w1[:, 0, t]
    brput("b1blk", 128, b1blk)

    w2 = np.asarray(inp["conv2_w"], np.float32)
    w2t = np.zeros((128, 384), np.float32)
    for t in range(KCONV):
        blk = w2[:, :, t].T
        w2t[0:64, 128 * t:128 * (t + 1)] = blk
        w2t[64:128, 128 * t:128 * (t + 1)] = blk
    brput("w2t", 128, w2t)

    w3 = np.asarray(inp["conv3_w"], np.float32)
    w3t = np.zeros((128, 384), np.float32)
    for t in range(KCONV):
        w3t[:, 128 * t:128 * (t + 1)] = w3[:, :, t].T
    brput("w3t", 128, w3t)

    def chunks2(a):     # [256, W] -> [128, 2W]
        a = np.asarray(a, np.float32)
        return np.concatenate([a[0:128], a[128:256]], axis=1)

    brput("g1w", 128, chunks2(inp["g1_w"]))
    brput("g2w", 128, chunks2(inp["g2_w"]))
    brput("th1w", 128, chunks2(inp["th1_w"]))
    brput("th2w", 128, chunks2(inp["th2_w"]))
    brput("fc2w", 128, chunks2(inp["fc2_w"]))
    brput("fc3w", 128, chunks2(inp["fc3_w"]))
    brput("fc4w", 128, np.pad(np.asarray(inp["fc4_w"], np.float32),
                              ((0, 0), (0, 3))))

    # selection matrices: B[k, 8jw+i] = sum_m Sel_jw[m, k] adjt[m, .]
    # blk1 (stride 1): Sel1_jw[8jw+j, 4jw+j] = 1    (40 x 32 each)
    sel1 = np.zeros((40, 32 * NW1), np.float32)
    for jw in range(NW1):
        for j in range(8):
            sel1[8 * jw + j, 32 * jw + 4 * jw + j] = 1.0
    brput("sel1", 40, sel1)
    # blk2 (stride 2): Sel2_jw[8jw+j, 8jw+j] = 1    (24 x 32 each)
    sel2 = np.zeros((24, 32 * NW2), np.float32)
    for jw in range(NW2):
        for j in range(8):
            sel2[8 * jw + j, 32 * jw + 8 * jw + j] = 1.0
    brput("sel2", 24, sel2)

    brput("eyen8", 8, np.eye(8, dtype=np.float32) * np.float32(-1e8))
    brput("ident8", 8, np.eye(8, dtype=np.float32))
    brput("identt", 8, np.tile(np.eye(8, dtype=np.float32), (1, NGT)))
    # graph extractors: gsel[rb + i, 8 g + i] = 1 pulls the diagonal block
    # of the full row-Gram into graph g's logit slot
    gsel = np.zeros((128, 8 * NGT), np.float32)
    for blk, (nw, stride, g0) in enumerate(((NW1, 1, 0), (NW2, 2, NG1))):
        for b in range(BSH):
            for jw in range(nw):
                g = g0 + nw * b + jw
                rb = 32 * b + 4 * stride * jw
                for i in range(8):
                    gsel[rb + i, 8 * g + i] = 1.0
    brput("gsel", 128, gsel)
    # +eye fold for B: E_jw[i, 4s jw + i] = 1  (adds Y[rb+i] to node i's msg)
    eyesel1 = np.zeros((8, 32 * NW1), np.float32)
    for jw in range(NW1):
        for i in range(8):
            eyesel1[i, 32 * jw + 4 * jw + i] = 1.0
    brput("eyesel1", 8, eyesel1)
    eyesel2 = np.zeros((8, 32 * NW2), np.float32)
    for jw in range(NW2):
        for i in range(8):
            eyesel2[i, 32 * jw + 8 * jw + i] = 1.0
    brput("eyesel2", 8, eyesel2)

    br = round_fp32r(br)

    bf = np.zeros((128, BF_COLS), np.float32)

    def bfput(name, rows, arr):
        c0, w = BF_LAY[name]
        a = np.asarray(arr, np.float32)
        assert a.shape == (rows, w), (name, a.shape)
        bf[0:rows, c0:c0 + w] = a

    s1, b1 = fold_bn(inp["bn1"])
    s2, b2 = fold_bn(inp["bn2"])
    s3, b3 = fold_bn(inp["bn3"])
    # pool-before-activation needs positive BN scales
    assert (s1 > 0).all() and (s2 > 0).all() and (s3 > 0).all()
    sa1, ba1 = fold_bn(inp["bnA1"])
    sa2, ba2 = fold_bn(inp["bnA2"])
    sm1f, bm1f = fold_bn(inp["bnM1"], extra_bias=inp["th1_b"], post_scale=0.5)
    sm2f, bm2f = fold_bn(inp["bnM2"], extra_bias=inp["th2_b"], post_scale=0.5)

    cv = np.zeros((128, NCV), np.float32)
    cv[0:64, C_S1] = s1
    cv[64:128, C_S1] = s1
    cv[0:64, C_B1] = b1
    cv[64:128, C_B1] = b1
    cv[:, C_S2], cv[:, C_B2] = s2, b2
    cv[:, C_S3], cv[:, C_B3] = s3, b3
    cv[:, C_SA1_0], cv[:, C_SA1_1] = sa1[0:128], sa1[128:256]
    cv[:, C_BA1_0], cv[:, C_BA1_1] = ba1[0:128], ba1[128:256]
    cv[:, C_SA2_0], cv[:, C_SA2_1] = sa2[0:128], sa2[128:256]
    cv[:, C_BA2_0], cv[:, C_BA2_1] = ba2[0:128], ba2[128:256]
    cv[:, C_SM1], cv[:, C_BM1] = sm1f, bm1f
    cv[:, C_SM2], cv[:, C_BM2] = sm2f, bm2f
    fb1 = np.asarray(inp["fc1_b"], np.float32)
    fb2 = np.asarray(inp["fc2_b"], np.float32)
    cv[:, C_FB1_0], cv[:, C_FB1_1] = fb1[0:128], fb1[128:256]
    cv[:, C_FB2_0], cv[:, C_FB2_1] = fb2[0:128], fb2[128:256]
    cv[:, C_FB3] = np.asarray(inp["fc3_b"], np.float32)
    g1b = np.asarray(inp["g1_b"], np.float32)
    g2b = np.asarray(inp["g2_b"], np.float32)
    cv[:, C_GB1_0], cv[:, C_GB1_1] = g1b[0:128], g1b[128:256]
    cv[:, C_GB2_0], cv[:, C_GB2_1] = g2b[0:128], g2b[128:256]
    cv[0:8, C_FB4] = np.pad(np.asarray(inp["fc4_b"], np.float32), (0, 3))
    bfput("cv", 128, cv)
    bfput("ident", 128, np.eye(128, dtype=np.float32))

    t_of = np.arange(8) // NNODE
    mask8 = DECAY ** np.abs(t_of[:, None] - t_of[None, :]).astype(np.float32)
    eye8 = np.eye(8, dtype=np.float32)
    bfput("mask8", 8, np.tile(mask8, (1, NGT)))
    bfput("fb4t", 8, np.tile(np.pad(np.asarray(inp["fc4_b"], np.float32),
                                    (0, 3))[:, None], (1, BSH)))

    g, be, m, v = (np.asarray(inp["bn_map2"][i], np.float64) for i in range(4))
    sc = r32(g / np.sqrt(v + 1e-5))
    bi = be + (np.asarray(inp["map2_b"], np.float64) - m) * (g / np.sqrt(v + 1e-5))
    # cv scale cols are per-partition = output channel chunks
    cv[:, C_SMAP_0], cv[:, C_SMAP_1] = sc[0:128], sc[128:256]
    bfput("cv", 128, cv)
    pe = pos_encoding().astype(np.float64)
    btot = np.zeros((R, D2), np.float64)
    for rr in range(R):
        btot[rr] = bi + pe[(rr // NNODE) % TLEN]
    btot = r32(btot)
    # transposed+padded bias table: btotT[c_part, 128h + 32b + k]
    btotT = np.zeros((128, D2), np.float32)
    for h in range(2):
        for b in range(BSH):
            for k in range(24):
                btotT[:, 128 * h + 32 * b + k] = btot[24 * b + k,
                                                      128 * h:128 * (h + 1)]
    bfput("btotT", 128, btotT)

    mw = np.asarray(inp["map2_w"], np.float32).reshape(LSTMO, CONV_OUT, D2)
    wprime = bf16(mw.transpose(1, 0, 2).reshape(FEAT_IN, D2))

    return {
        "blob_r": br, "blob_f": bf, "wprime": wprime,
        "fc1w": bf16(inp["fc1_w"]),
        "w2tb": bf16(np.concatenate(
            [w2t, chunks2(inp["g1_w"]), chunks2(inp["g2_w"]),
             chunks2(inp["th1_w"]), chunks2(inp["th2_w"])], axis=1)),
    }


def _mm(nc, out, lhsT, rhs, **kw):
    if lhsT.dtype == F32:
        lhsT = lhsT.bitcast(F32R)
    if rhs.dtype == F32:
        rhs = rhs.bitcast(F32R)
    nc.tensor.matmul(out, lhsT, rhs, **kw)


def build_program():
    nc = bacc.Bacc("TRN2", target_bir_lowering=False, debug=False)

    t1all_d = nc.dram_tensor("t1all", [3 * R, 1502], F32R, kind="ExternalInput")
    br_d = nc.dram_tensor("blob_r", [128, BR_COLS], F32R, kind="ExternalInput")
    bf_d = nc.dram_tensor("blob_f", [128, BF_COLS], F32, kind="ExternalInput")
    wp_d = nc.dram_tensor("wprime", [FEAT_IN, D2], BF16, kind="ExternalInput")
    w2b_d = nc.dram_tensor("w2tb", [128, 1920], BF16, kind="ExternalInput")
    f1_d = nc.dram_tensor("fc1w", [4096, D2], BF16, kind="ExternalInput")
    out_d = nc.dram_tensor("out", [BSH, 5], F32, kind="ExternalOutput")

    with tile.TileContext(nc) as tc, ExitStack() as st:
        persist = st.enter_context(tc.tile_pool(name="persist", bufs=1))

        # conv-critical consts first; everything else queued behind them
        brsb = persist.tile([128, BR_COLS], F32R)
        bfsb = persist.tile([128, BF_COLS], F32)
        nc.sync.dma_start(brsb[:, 0:128], br_d[:, 0:128])
        w2sb = persist.tile([128, 1920], BF16)
        nc.sync.dma_start(w2sb[:, 0:384], w2b_d[:, 0:384])
        nc.sync.dma_start(bfsb[:, 0:BF_CONV], bf_d[:, 0:BF_CONV])
        nc.sync.dma_start(brsb[:, 128:BR_CONV], br_d[:, 128:BR_CONV])

        def brs(name, rows=128):            # fp32r blob slice
            c0, w = BR_LAY[name]
            return brsb[0:rows, c0:c0 + w]

        def bfs(name, rows=128):            # fp32 blob slice
            c0, w = BF_LAY[name]
            return bfsb[0:rows, c0:c0 + w]

        def cvcol(i):
            c0, _ = BF_LAY["cv"]
            return bfsb[:, c0 + i:c0 + i + 1]

        identsb = bfs("ident")

        a3 = persist.tile([128, R * P3], BF16)          # conv3 out, [o, r*191+l]
        a3v = a3.rearrange("p (r l) -> p r l", l=P3)

        # manual rings with one-time pad zeroing (pads are read by tap slides)
        p1a_ring = [persist.tile([128, 771], BF16, name=f"p1a{i}")
                    for i in range(4)]
        s2a_ring = [persist.tile([128, 384], F32R, name=f"s2a{i}")
                    for i in range(5)]
        for t_ in p1a_ring:
            nc.vector.memset(t_[:, 0:2], 0.0)
            nc.vector.memset(t_[:, 753:771], 0.0)
        for t_ in s2a_ring:
            nc.vector.memset(t_[:, 0:3].bitcast(F32), 0.0)
            nc.vector.memset(t_[:, 380:384].bitcast(F32), 0.0)

        # ======== wprime ring (bf16), lives through conv + map2 ========
        wring_cm = tc.tile_pool(name="wring", bufs=N_WRING)
        wring = wring_cm.__enter__()
        wtiles = {}

        def wp_load(wi):
            l0 = W_TILE * wi
            nch = min(W_TILE, P3 - l0)
            wt = wring.tile([128, W_TILE * D2], BF16, tag="wp")
            nc.sync.dma_start(
                wt.rearrange("p (n c) -> p n c", c=D2)[:, 0:nch, :],
                wp_d[128 * l0:128 * (l0 + nch), :]
                .rearrange("(n p) c -> p n c", p=128),
            )
            wtiles[wi] = wt

        # ======== conv stack ========
        # engine split: ACT does the two big BN+ReLUs (e1, e2); Pool engine
        # pools conv1 + conv3 (SBUF-side); DVE pools conv2 and does conv3's
        # scale+bias (relu folds into the pool's max-with-0).
        with (
            tc.tile_pool(name="t1", bufs=2) as t1p,
            tc.tile_pool(name="e1", bufs=3) as e1p,
            tc.tile_pool(name="e2", bufs=2) as e2p,
            tc.tile_pool(name="e3", bufs=3) as e3p,
            tc.tile_pool(name="psc1", bufs=1, space="PSUM") as psc1,
            tc.tile_pool(name="psc2", bufs=2, space="PSUM") as psc2,
            tc.tile_pool(name="psc3", bufs=1, space="PSUM") as psc3,
        ):
            def t1_load(pg):
                t1 = t1p.tile([6, PGRP * 1502], F32R, name=f"t1g")
                nc.gpsimd.dma_start(
                    t1.rearrange("k (pl c) -> k pl c", c=1502)[:],
                    t1all_d[6 * PGRP * pg:6 * PGRP * (pg + 1), :]
                    .rearrange("(pl k) c -> k pl c", k=6),
                )
                return t1

            NGROUPS = PAIRS // PGRP
            t1tiles = {0: t1_load(0)}

            def stage_a(p):
                # conv1 of pair p (+ group-boundary prefetches)
                pg, pl = divmod(p, PGRP)
                if pl == 0:
                    if pg + 1 < NGROUPS:
                        t1tiles[pg + 1] = t1_load(pg + 1)
                    if pg < N_WRING:
                        wp_load(pg)
                    if pg == 1:
                        nc.sync.dma_start(brsb[:, BR_CONV:], br_d[:, BR_CONV:])
                        nc.sync.dma_start(w2sb[:, 384:], w2b_d[:, 384:])
                        nc.sync.dma_start(bfsb[:, BF_CONV:], bf_d[:, BF_CONV:])
                t1 = t1tiles[pg]
                bc = 1502 * pl
                ps1 = psc1.tile([128, L1], F32)
                for c0, c1 in ((0, 512), (512, 1024), (1024, L1)):
                    _mm(nc, ps1[:, c0:c1], brs("b1blk", 6),
                        t1[:, bc + c0:bc + c1])
                e1 = e1p.tile([128, L1], F32)
                nc.scalar.activation(e1[:], ps1[:], ACTF.Relu,
                                     bias=cvcol(C_B1), scale=cvcol(C_S1))
                pa = p1a_ring[p % 4]
                nc.vector.tensor_max(
                    pa[:, 3:752], e1[:, 1:1498:2], e1[:, 2:1499:2])
                nc.vector.tensor_copy(
                    pa[:, 2:753:750], e1[:, 0:L1:L1 - 1])
                if pl == PGRP - 1:
                    t1tiles.pop(pg)

            def stage_b(p):
                # conv2 of pair p's two rows (matmuls first, then epilogues)
                pa = p1a_ring[p % 4]
                ps2s = []
                for ri in range(2):
                    base = 64 * ri
                    ps2 = psc2.tile([128, 753], F32)
                    for t in range(KCONV):
                        lhs = w2sb[base:base + 64, 128 * t:128 * (t + 1)]
                        _mm(nc, ps2[:, 0:512], lhs,
                            pa[base:base + 64, t:t + 512],
                            start=(t == 0), stop=(t == 2))
                        _mm(nc, ps2[:, 512:753], lhs,
                            pa[base:base + 64, t + 512:t + 753],
                            start=(t == 0), stop=(t == 2))
                    ps2s.append(ps2)
                for ri in range(2):
                    r = 2 * p + ri
                    ps2 = ps2s[ri]
                    e2 = e2p.tile([128, L2], F32)
                    sa_ = s2a_ring[r % 5]
                    if r % 6 == 5:
                        # DVE-only path (ACT relief): scale+bias without
                        # relu, then fold relu into the pooling max
                        nc.vector.tensor_scalar(e2[:], ps2[:, 0:L2],
                                                cvcol(C_S2), cvcol(C_B2),
                                                op0=AL.mult, op1=AL.add)
                        nc.vector.scalar_tensor_tensor(
                            sa_[:, 4:380], e2[:, 1:752:2], 0.0,
                            e2[:, 2:753:2], op0=AL.max, op1=AL.max)
                        nc.vector.tensor_scalar_max(sa_[:, 3:4],
                                                    e2[:, 0:1], 0.0)
                    else:
                        nc.scalar.activation(e2[:], ps2[:, 0:L2],
                                             ACTF.Relu, bias=cvcol(C_B2),
                                             scale=cvcol(C_S2))
                        nc.vector.tensor_max(
                            sa_[:, 4:380], e2[:, 1:752:2], e2[:, 2:753:2])
                        nc.vector.tensor_copy(sa_[:, 3:4], e2[:, 0:1])

            def stage_c(r):
                # conv3 + pooling of one row
                sa_ = s2a_ring[r % 5]
                ps3 = psc3.tile([128, L3 + 1], F32)
                for t in range(KCONV):
                    _mm(nc, ps3[:], brs("w3t")[:, 128 * t:128 * (t + 1)],
                        sa_[:, t:t + L3 + 1], start=(t == 0), stop=(t == 2))
                e3 = e3p.tile([128, L3], F32)
                if r % 2 == 0:
                    nc.scalar.activation(e3[:], ps3[:, 0:L3], ACTF.Relu,
                                         bias=cvcol(C_B3), scale=cvcol(C_S3))
                    nc.vector.tensor_max(
                        a3v[:, r, 1:P3], e3[:, 1:380:2], e3[:, 2:381:2])
                    nc.vector.tensor_copy(a3v[:, r, 0:1], e3[:, 0:1])
                else:
                    nc.vector.tensor_scalar(e3[:], ps3[:, 0:L3],
                                            cvcol(C_S3), cvcol(C_B3),
                                            op0=AL.mult, op1=AL.add)
                    nc.vector.scalar_tensor_tensor(
                        a3v[:, r, 1:P3], e3[:, 1:380:2], 0.0,
                        e3[:, 2:381:2], op0=AL.max, op1=AL.max)
                    nc.vector.tensor_scalar_max(a3v[:, r, 0:1],
                                                e3[:, 0:1], 0.0)

            # 4-stage software pipeline: every cross-engine hand-off gets a
            # full tick (~3.5us of PE work) of slack. The last pair's odd
            # row is pulled one tick forward to shorten the drain.
            for i in range(PAIRS + 2):
                if i < PAIRS:
                    stage_a(i)
                if 1 <= i <= PAIRS:
                    stage_b(i - 1)
                if 2 <= i <= PAIRS + 1:
                    stage_c(2 * (i - 2))
                if 3 <= i <= PAIRS:
                    stage_c(2 * (i - 3) + 1)
                if i == PAIRS + 1:
                    stage_c(2 * (PAIRS - 2) + 1)
                    stage_c(2 * (PAIRS - 1) + 1)

        # ======== map2 (bf16 stream) ========
        araw = persist.tile([R, D2], F32)
        with tc.tile_pool(name="mp2", bufs=1, space="PSUM") as mp2:
            psm = mp2.tile([R, D2], F32)
            for wi in range(N_WTILES):
                if wi not in wtiles:
                    wp_load(wi)
                wt = wtiles.pop(wi)
                l0 = W_TILE * wi
                nch = min(W_TILE, P3 - l0)
                for k in range(nch):
                    l = l0 + k
                    _mm(nc, psm[:], a3v[:, :, l], wt[:, D2 * k:D2 * (k + 1)],
                        start=(l == 0), stop=(l == P3 - 1))
            nc.vector.tensor_copy(araw[:], psm[:])
        wring_cm.__exit__(None, None, None)

        # prefetch fc1 chunks now (bf16, 2MB) so the FC head never stalls
        fring_cm = tc.tile_pool(name="fring", bufs=FC_CHUNKS)
        fring = fring_cm.__enter__()
        wcs = []
        for ci in range(FC_CHUNKS):
            wc = fring.tile([128, 4 * D2], BF16, tag="f1w")
            nc.sync.dma_start(
                wc.rearrange("p (n c) -> p n c", c=D2)[:],
                f1_d[512 * ci:512 * (ci + 1), :]
                .rearrange("(n p) c -> p n c", p=128),
            )
            wcs.append(wc)

        # ======== MPNN blocks (batch-padded row layout r' = 32b + r%24) ====
        o1 = persist.tile([128, NNODE * NG1], BF16)   # block out^T [c,(b,jw,n)]
        o2 = persist.tile([128, NNODE * NG2], BF16)
        with (
            tc.tile_pool(name="mp_sb", bufs=2) as msb,
            tc.tile_pool(name="mp_ps", bufs=1, space="PSUM") as mps,
            tc.tile_pool(name="mp_tp", bufs=2, space="PSUM") as mtp,
            tc.tile_pool(name="mp_tpa", bufs=1, space="PSUM") as mtpa,
            tc.tile_pool(name="mp_bp", bufs=1, space="PSUM") as mbp,
        ):
            # aft_pad[h]: [128, 128] fp32r, cols r' = 32b + (r % 24), pads 0
            # BN(map2) applied transposed: per-partition scale + btotT table
            aft = []
            for h in range(2):
                pt = mtp.tile([128, R], F32, tag="tp")
                nc.tensor.transpose(pt[:], araw[:, 128 * h:128 * (h + 1)],
                                    identsb[0:R, 0:R])
                t_ = persist.tile([128, 128], BF16, tag=f"aftp{h}")
                nc.vector.memset(t_[:], 0.0)
                # all 4 batch quads in one op: per-operand strides differ
                # (dst cols 32b+k, src cols 24b+k, table cols 128h+32b+k)
                nc.vector.scalar_tensor_tensor(
                    t_.rearrange("p (b k) -> p b k", k=32)[:, :, 0:24],
                    pt.rearrange("p (b k) -> p b k", k=24)[:],
                    cvcol(C_SMAP_0 + h),
                    bfs("btotT")[:, 128 * h:128 * (h + 1)]
                    .rearrange("p (b k) -> p b k", k=32)[:, :, 0:24],
                    op0=AL.mult, op1=AL.add)
                aft.append(t_)

            BLK = [  # (G, nw, stride, gcols0) per block; blk2 cols follow blk1
                (NG1, NW1, 1, 0),
                (NG2, NW2, 2, NG1),
            ]

            def blkc(blk, lst):
                return lst[blk]

            # --- phase A: graph logits for BOTH blocks into one psum tile
            awall = mps.tile([8, 8 * NGT], F32, tag="awall")
            # -1e8*eye pre-fill of every graph's logits in one matmul
            _mm(nc, awall[:], brs("eyen8", 8), brs("identt", 8),
                start=True, stop=False)
            for blk in range(2):
                G, nw, stride, g0 = BLK[blk]
                gw = blkc(blk, (w2sb[:, 384:896], w2sb[:, 896:1408]))
                gbc = blkc(blk, (C_GB1_0, C_GB2_0))
                ft = []
                for h in range(2):
                    ftp = mps.tile([128, 128], F32, tag="ftp")
                    for dh in range(2):
                        _mm(nc, ftp[:],
                            gw[:, 256 * dh + 128 * h:256 * dh + 128 * h + 128],
                            aft[dh][:], start=(dh == 0), stop=(dh == 1))
                    t_ = msb.tile([128, 128], F32R, tag=f"ft{h}")
                    nc.scalar.activation(t_[:], ftp[:], ACTF.Identity,
                                         bias=cvcol(gbc + h), scale=1.0)
                    ft.append(t_)
                for b in range(BSH):
                    for jw in range(nw):
                        g = g0 + nw * b + jw
                        rb = 32 * b + 4 * stride * jw
                        for h in range(2):
                            _mm(nc, awall[:, 8 * g:8 * (g + 1)],
                                ft[h][:, rb:rb + 8], ft[h][:, rb:rb + 8],
                                start=False, stop=(h == 1))

            # --- one softmax/mask chain over all 32 graphs
            # Adj = softmax(lrelu(L - 1e8 eye)) * mask + eye   (rows = i)
            NGA = 8 * NGT
            aw2 = msb.tile([8, NGA], F32, tag="aw2")
            aw3 = msb.tile([8, NGA], F32, tag="aw3")
            aw4 = msb.tile([8, NGA], F32, tag="aw4")
            aw5 = msb.tile([8, NGA], F32, tag="aw5")
            aw6 = msb.tile([8, NGA], F32, tag="aw6")
            rmax = msb.tile([8, NGT], F32, tag="rmax")
            rsum = msb.tile([8, NGT], F32, tag="rsum")
            rrec = msb.tile([8, NGT], F32, tag="rrec")
            nc.scalar.activation(aw2[:], awall[:], ACTF.Lrelu, alpha=0.01)
            a3d = aw2.rearrange("p (g j) -> p g j", j=8)
            nc.vector.reduce_max(rmax[:], a3d[:], axis=mybir.AxisListType.X)
            nc.vector.tensor_sub(aw3.rearrange("p (g j) -> p g j", j=8)[:],
                                 a3d[:],
                                 rmax[:].unsqueeze(2).broadcast_to([8, NGT, 8]))
            nc.scalar.activation(aw4[:], aw3[:], ACTF.Exp)
            nc.vector.reduce_sum(rsum[:],
                                 aw4.rearrange("p (g j) -> p g j", j=8)[:],
                                 axis=mybir.AxisListType.X)
            nc.vector.reciprocal(rrec[:], rsum[:])
            nc.vector.tensor_mul(aw5.rearrange("p (g j) -> p g j", j=8)[:],
                                 aw4.rearrange("p (g j) -> p g j", j=8)[:],
                                 rrec[:].unsqueeze(2).broadcast_to([8, NGT, 8]))
            nc.vector.tensor_mul(aw6[:], aw5[:], bfs("mask8", 8))

            # --- phase B: adjacency scatter + message passing, both blocks
            pshall = mps.tile([128, 8 * NGT], F32, tag="pshall")
            for blk in range(2):
                G, nw, stride, g0 = BLK[blk]
                thw = blkc(blk, (w2sb[:, 1408:1664], w2sb[:, 1664:1920]))
                sel = blkc(blk, (brs("sel1", 40), brs("sel2", 24)))
                eyesel = blkc(blk, (brs("eyesel1", 8), brs("eyesel2", 8)))
                nsel = blkc(blk, (40, 24))
                sa = blkc(blk, ((C_SA1_0, C_SA1_1), (C_SA2_0, C_SA2_1)))
                ba = blkc(blk, ((C_BA1_0, C_BA1_1), (C_BA2_0, C_BA2_1)))

                # Adj^T per batch quad: adjt_all[8jw+j, 8b+i] = Adj_g[i, j]
                pta = mtpa.tile([nsel, 32], F32, tag="tpa")
                for b in range(BSH):
                    nc.tensor.transpose(
                        pta[:, 8 * b:8 * (b + 1)],
                        aw6[:, 8 * (g0 + nw * b):8 * (g0 + nw * (b + 1))],
                        identsb[0:8, 0:8])
                adjt = msb.tile([nsel, 32], F32R, tag="adjt")
                nc.vector.tensor_copy(adjt[:], pta[:])

                # banded B quads (partitions 0:32, col-chunked per batch):
                #   B_b[k, 8jw+i] = sum_m Sel_jw[m, k] adjt[m, 8b+i]
                bps = mbp.tile([32, 8 * nw * BSH], F32, tag="bps")
                for b in range(BSH):
                    for jw in range(nw):
                        c0 = 8 * nw * b + 8 * jw
                        _mm(nc, bps[0:32, c0:c0 + 8],
                            eyesel[:, 32 * jw:32 * (jw + 1)],
                            brs("ident8", 8), start=True, stop=False)
                        _mm(nc, bps[0:32, c0:c0 + 8],
                            sel[:, 32 * jw:32 * (jw + 1)],
                            adjt[:, 8 * b:8 * (b + 1)],
                            start=False, stop=True)
                bsb = msb.tile([32, 8 * nw * BSH], F32R, tag="bsb")
                nc.vector.tensor_copy(bsb[:], bps[:])

                # Xb^T = BN_A(AF^T); Y_pad = Xb @ thw  [128 r', 128]
                xbt = []
                for h in range(2):
                    t_ = msb.tile([128, 128], BF16, tag=f"xbt{h}")
                    nc.scalar.activation(t_[:], aft[h][:], ACTF.Identity,
                                         bias=cvcol(ba[h]), scale=cvcol(sa[h]))
                    xbt.append(t_)
                psy = mps.tile([128, HID], F32, tag="psy")
                for h in range(2):
                    _mm(nc, psy[:], xbt[h][:], thw[:, HID * h:HID * (h + 1)],
                        start=(h == 0), stop=(h == 1))
                # Y quads down to partitions 0:32 (col-chunked per batch)
                ysbq = msb.tile([32, HID * BSH], F32R, tag="ysbq")
                for b in range(BSH):
                    nc.vector.tensor_copy(ysbq[0:32, HID * b:HID * (b + 1)],
                                          psy[32 * b:32 * (b + 1), :])

                # h^T: psh[c, 8(g0 + nw b + jw) + i] via 4 K=32 matmuls
                for b in range(BSH):
                    c0 = 8 * (g0 + nw * b)
                    _mm(nc, pshall[:, c0:c0 + 8 * nw],
                        ysbq[0:32, HID * b:HID * (b + 1)],
                        bsb[0:32, 8 * nw * b:8 * nw * (b + 1)])

            # lrelu(psh * sM + bM)  (incl th_b, BN_M, x0.5 mean-fold)
            hp = msb.tile([128, 8 * NGT], F32, tag="hp")
            nc.scalar.activation(hp[:, 0:8 * NG1], pshall[:, 0:8 * NG1],
                                 ACTF.Lrelu, alpha=0.01,
                                 bias=cvcol(C_BM1), scale=cvcol(C_SM1))
            nc.scalar.activation(hp[:, 8 * NG1:], pshall[:, 8 * NG1:],
                                 ACTF.Lrelu, alpha=0.01,
                                 bias=cvcol(C_BM2), scale=cvcol(C_SM2))
            hpv = hp.rearrange("p (g j) -> p g j", j=8)
            nc.vector.tensor_add(o1.rearrange("p (g n) -> p g n", n=4)[:],
                                 hpv[:, 0:NG1, 0:4], hpv[:, 0:NG1, 4:8])
            nc.vector.tensor_add(o2.rearrange("p (g n) -> p g n", n=4)[:],
                                 hpv[:, NG1:NGT, 0:4], hpv[:, NG1:NGT, 4:8])

        # ======== FC head (transposed throughout) ========
        with (
            tc.tile_pool(name="fc_sb", bufs=2) as fsb_p,
            tc.tile_pool(name="fc_ps", bufs=1, space="PSUM") as fps,
        ):
            o1v = o1.rearrange("p (b k) -> p b k", k=4 * NW1)
            o2v = o2.rearrange("p (b k) -> p b k", k=4 * NW2)
            pst = [fps.tile([128, BSH], F32, name=f"pst1_{h}", tag=f"p1_{h}")
                   for h in range(2)]
            ci = 0
            npairs = (NW1 + NW2) * NNODE             # 32 accumulation steps
            for njw, o_v, ci0 in ((NW1, o1v, 0), (NW2, o2v, NW1)):
                for jw in range(njw):
                    wc = wcs[ci0 + jw]
                    wcv = wc.rearrange("p (n c) -> p n c", c=D2)
                    for n in range(NNODE):
                        for h in range(2):
                            _mm(nc, pst[h][:],
                                wcv[:, n, 128 * h:128 * (h + 1)],
                                o_v[:, :, 4 * jw + n],
                                start=(ci == 0), stop=(ci == npairs - 1))
                        ci += 1

            def relu_t(ps_list, bias_cols):
                outs = []
                for h, ps_ in enumerate(ps_list):
                    t_ = fsb_p.tile([128, BSH], F32R, tag=f"h{bias_cols[h]}")
                    nc.scalar.activation(t_[:], ps_[:], ACTF.Relu,
                                         bias=cvcol(bias_cols[h]), scale=1.0)
                    outs.append(t_)
                return outs

            h1t = relu_t(pst, (C_FB1_0, C_FB1_1))
            pst2 = [fps.tile([128, BSH], F32, name=f"pst2_{h}", tag=f"p2_{h}")
                    for h in range(2)]
            for ch in range(2):
                for dh in range(2):
                    _mm(nc, pst2[ch][:],
                        brs("fc2w")[:, 256 * dh + 128 * ch:256 * dh + 128 * ch + 128],
                        h1t[dh][:], start=(dh == 0), stop=(dh == 1))
            h2t = relu_t(pst2, (C_FB2_0, C_FB2_1))
            pst3 = fps.tile([128, BSH], F32, tag="p3")
            for dh in range(2):
                _mm(nc, pst3[:], brs("fc3w")[:, 128 * dh:128 * dh + 128],
                    h2t[dh][:], start=(dh == 0), stop=(dh == 1))
            h3t = relu_t([pst3], (C_FB3,))
            pst4 = fps.tile([8, BSH], F32, tag="p4")
            _mm(nc, pst4[:], brs("fc4w"), h3t[0][:])
            osb = fsb_p.tile([8, BSH], F32, tag="osb")
            nc.scalar.activation(osb[:], pst4[:], ACTF.Identity,
                                 bias=cvcol(C_FB4)[0:8, :])
            # transpose on the way out: out[b, j] = osb[j, b]
            dst = dataclasses.replace(out_d[:], ap=[[1, 5], [5, 4]], offset=0)
            nc.gpsimd.dma_start(dst, osb[0:5, :])

        fring_cm.__exit__(None, None, None)

    nc.compile()
    return nc


_CACHE = {}


def _get_program():
    if "nc" not in _CACHE:
        _CACHE["nc"] = build_program()
    return _CACHE["nc"]


def make_in_maps(inputs):
    consts = prep_consts(inputs)
    x = np.asarray(inputs["X"], np.float32).reshape(BS * TLEN * NNODE, DIM)
    in_maps = []
    for c in range(NCORES):
        shard = x[R * c:R * (c + 1)]
        xp = np.zeros((R, 1504), np.float32)
        xp[:, 1:1 + DIM] = shard
        # t1all[3r + t, c] = xpad[r, c + t]
        sw = np.lib.stride_tricks.sliding_window_view(xp, 1502, axis=1)[:, 0:3]
        t1all = round_fp32r(sw.reshape(3 * R, 1502))
        m = {"t1all": t1all}
        m.update(consts)
        in_maps.append(m)
    return in_maps


def kernel(**inputs):
    from concourse.bass_utils import run_bass_kernel_spmd

    nc = _get_program()
    in_maps = make_in_maps(inputs)
    res = run_bass_kernel_spmd(nc, in_maps, core_ids=list(range(NCORES)))
    outs = [np.asarray(res.results[c]["out"]) for c in range(NCORES)]
    return np.concatenate(outs, axis=0).astype(np.float32)


# revision 47
# speedup vs baseline: 1.0032x; 1.0032x over previous
"""Trainium2 Bass kernel for nn_FC_STGNN_SSC (STGNN over conv sleep-features).

Data-parallel over the batch: each of 8 NeuronCores processes 4 of the 32
batch elements (96 of the 768 flattened conv rows). All weights replicated.

Per-core pipeline (PE matmuls in fp32r; map2/fc1 streams in bf16):
  conv stack as a 4-stage software pipeline (conv1 of pair i, conv2 of pair
  i-1, conv3-even of pair i-2, conv3-odd of pair i-3) so every cross-engine
  hand-off has a full ~3.5us tick of slack; epilogues are engine-balanced
  (ACT does e1/e2 BN+ReLU, DVE pools conv2 + conv3 and runs 1-in-6 conv2
  rows + conv3-odd rows as scale/bias with relu folded into the pooling max).
  map2 24448->256: bf16 W' streamed through a 12-deep SBUF ring (PE-bound);
    a3 kept bf16; BN+posenc applied transposed after the PE transpose.
  MPNN x2: batch-padded row layout (32 rows/batch quad); F^T built directly
    as gw^T @ AF^T matmuls with per-partition bias; -1e8*eye pre-filled into
    the Gram PSUM by one const matmul; +eye and the adjacency scatter done
    by tiny PE matmuls against constant 0/1 selectors into a banded B;
    message passing = 4 K=32 matmuls per block at tile position (0,0);
    lrelu chains fused into single Lrelu activations.
  FC head fully transposed (h^T [c,4] tiles): no PSUM->SBUF round trips,
    biases folded into per-partition ACT; fc1 weights prefetched after map2;
    output transposed on the way out by a custom-AP DMA.
"""

import dataclasses
import os
import sys
from contextlib import ExitStack

import numpy as np

if not any(os.path.isdir(os.path.join(p, "concourse")) for p in sys.path if p):
    sys.path.insert(0, "/opt/trn_rl_repo")

import ml_dtypes  # noqa: E402

import concourse.bass as bass  # noqa: E402,F401
import concourse.bacc as bacc  # noqa: E402
import concourse.mybir as mybir  # noqa: E402
import concourse.tile as tile  # noqa: E402

F32 = mybir.dt.float32
F32R = mybir.dt.float32r
BF16 = mybir.dt.bfloat16
ACTF = mybir.ActivationFunctionType
AL = mybir.AluOpType

# model dims
BS, TLEN, NNODE, DIM = 32, 6, 4, 1500
HID = 128
D2 = 256
LSTMH, LSTMO, KCONV = 64, 128, 3
CONV_OUT = 191
DECAY = 0.7
FEAT_IN = LSTMO * CONV_OUT  # 24448

NCORES = 8
BSH = BS // NCORES          # 4 batch elems per core
R = BSH * TLEN * NNODE      # 96 conv rows per core
PAIRS = R // 2              # 48
PGRP = 4                    # conv1 pairs per T1 tile

L1, P1 = 1500, 751
L2, P2 = 753, 377
L3, P3 = 381, 191

NW1, NW2 = 5, 3             # windows per batch elem (stride 1 / 2)
NG1, NG2 = BSH * NW1, BSH * NW2   # 20, 12 graphs
NGT = NG1 + NG2             # 32 graphs over both MPNN blocks

# const-vector column indices within the cv section
(C_S1, C_B1, C_S2, C_B2, C_S3, C_B3,
 C_SA1_0, C_SA1_1, C_BA1_0, C_BA1_1,
 C_SA2_0, C_SA2_1, C_BA2_0, C_BA2_1,
 C_SM1, C_BM1, C_SM2, C_BM2,
 C_FB1_0, C_FB1_1, C_FB2_0, C_FB2_1, C_FB3,
 C_SMAP_0, C_SMAP_1, C_GB1_0, C_GB1_1, C_GB2_0, C_GB2_1, C_FB4,
 NCV) = range(31)


# ---- const blob layouts: name -> (col offset, ncols)
def _mk_layout(sections):
    off, lay = 0, {}
    for name, w in sections:
        lay[name] = (off, w)
        off += w
    return lay, off


# conv weights first so the first (small) blob DMA unblocks conv1 quickly
BR_LAY, BR_COLS = _mk_layout([
    ("b1blk", 128), ("w2t", 384), ("w3t", 384),
    ("g1w", 512), ("g2w", 512), ("th1w", 256), ("th2w", 256),
    ("fc2w", 512), ("fc3w", 256), ("fc4w", 8),
    ("sel1", 32 * NW1), ("sel2", 32 * NW2),
    ("eyen8", 8), ("eyesel1", 32 * NW1), ("eyesel2", 32 * NW2),
    ("ident8", 8), ("identt", 8 * NGT), ("gsel", 8 * NGT),
])
BR_CONV = 896  # cols holding conv weights (first DMAs)
BF_LAY, BF_COLS = _mk_layout([
    ("cv", NCV), ("ident", 128), ("mask8", 8 * NGT),
    ("fb4t", BSH), ("btotT", D2),
])
BF_CONV = NCV  # cols needed by the conv stack (first bfsb DMA)

W_TILE = 8          # wprime l-chunks per DMA
N_WTILES = (P3 + W_TILE - 1) // W_TILE   # 24
N_WRING = 12
FC_CHUNKS = 8       # fc1 512-row chunks


def r32(x):
    return np.ascontiguousarray(x, dtype=np.float32)


def bf16(x):
    return np.asarray(x, np.float32).astype(ml_dtypes.bfloat16)


def round_fp32r(x):
    """fp32 -> fp32r (11-bit mantissa, RNE); matches walrus fp32_to_fp32r."""
    u = np.ascontiguousarray(x, np.float32).view(np.uint32).astype(np.uint64)
    r = ((u + 0x7FF + ((u >> 12) & 1)) & 0xFFFFF000).astype(np.uint32)
    return r.view(np.float32).reshape(np.shape(x))


def fold_bn(p, extra_bias=None, post_scale=1.0):
    """y = x*scale + bias  ==  post_scale * BN(x + extra_bias)."""
    g, be, m, v = (np.asarray(p[i], np.float64) for i in range(4))
    sc = g / np.sqrt(v + 1e-5)
    bi = be - m * sc
    if extra_bias is not None:
        bi = bi + np.asarray(extra_bias, np.float64) * sc
    return r32(sc * post_scale), r32(bi * post_scale)


def pos_encoding():
    pos = np.arange(TLEN, dtype=np.float32)[:, None]
    div = np.exp(np.arange(0, D2, 2, dtype=np.float32)
                 * (np.float32(-np.log(np.float32(100.0))) / np.float32(D2)))
    pe = np.zeros((TLEN, D2), np.float32)
    pe[:, 0::2] = np.sin(pos * div)
    pe[:, 1::2] = np.cos(pos * div)
    return pe


def prep_consts(inp):
    """Host-side constant prep (shared by all cores)."""
    br = np.zeros((128, BR_COLS), np.float32)

    def brput(name, rows, arr):
        c0, w = BR_LAY[name]
        a = np.asarray(arr, np.float32)
        assert a.shape == (rows, w), (name, a.shape)
        br[0:rows, c0:c0 + w] = a

    w1 = np.asarray(inp["conv1_w"], np.float32)
    b1blk = np.zeros((128, 128), np.float32)
    for ri in range(2):
        for t in range(KCONV):
            b1blk[3 * ri + t, 64 * ri:64 * ri + 64] = w1[:, 0, t]
    brput("b1blk", 128, b1blk)

    w2 = np.asarray(inp["conv2_w"], np.float32)
    w2t = np.zeros((128, 384), np.float32)
    for t in range(KCONV):
        blk = w2[:, :, t].T
        w2t[0:64, 128 * t:128 * (t + 1)] = blk
        w2t[64:128, 128 * t:128 * (t + 1)] = blk
    brput("w2t", 128, w2t)

    w3 = np.asarray(inp["conv3_w"], np.float32)
    w3t = np.zeros((128, 384), np.float32)
    for t in range(KCONV):
        w3t[:, 128 * t:128 * (t + 1)] = w3[:, :, t].T
    brput("w3t", 128, w3t)

    def chunks2(a):     # [256, W] -> [128, 2W]
        a = np.asarray(a, np.float32)
        return np.concatenate([a[0:128], a[128:256]], axis=1)

    brput("g1w", 128, chunks2(inp["g1_w"]))
    brput("g2w", 128, chunks2(inp["g2_w"]))
    brput("th1w", 128, chunks2(inp["th1_w"]))
    brput("th2w", 128, chunks2(inp["th2_w"]))
    brput("fc2w", 128, chunks2(inp["fc2_w"]))
    brput("fc3w", 128, chunks2(inp["fc3_w"]))
    brput("fc4w", 128, np.pad(np.asarray(inp["fc4_w"], np.float32),
                              ((0, 0), (0, 3))))

    # selection matrices: B[k, 8jw+i] = sum_m Sel_jw[m, k] adjt[m, .]
    # blk1 (stride 1): Sel1_jw[8jw+j, 4jw+j] = 1    (40 x 32 each)
    sel1 = np.zeros((40, 32 * NW1), np.float32)
    for jw in range(NW1):
        for j in range(8):
            sel1[8 * jw + j, 32 * jw + 4 * jw + j] = 1.0
    brput("sel1", 40, sel1)
    # blk2 (stride 2): Sel2_jw[8jw+j, 8jw+j] = 1    (24 x 32 each)
    sel2 = np.zeros((24, 32 * NW2), np.float32)
    for jw in range(NW2):
        for j in range(8):
            sel2[8 * jw + j, 32 * jw + 8 * jw + j] = 1.0
    brput("sel2", 24, sel2)

    brput("eyen8", 8, np.eye(8, dtype=np.float32) * np.float32(-1e8))
    brput("ident8", 8, np.eye(8, dtype=np.float32))
    brput("identt", 8, np.tile(np.eye(8, dtype=np.float32), (1, NGT)))
    # graph extractors: gsel[rb + i, 8 g + i] = 1 pulls the diagonal block
    # of the full row-Gram into graph g's logit slot
    gsel = np.zeros((128, 8 * NGT), np.float32)
    for blk, (nw, stride, g0) in enumerate(((NW1, 1, 0), (NW2, 2, NG1))):
        for b in range(BSH):
            for jw in range(nw):
                g = g0 + nw * b + jw
                rb = 32 * b + 4 * stride * jw
                for i in range(8):
                    gsel[rb + i, 8 * g + i] = 1.0
    brput("gsel", 128, gsel)
    # +eye fold for B: E_jw[i, 4s jw + i] = 1  (adds Y[rb+i] to node i's msg)
    eyesel1 = np.zeros((8, 32 * NW1), np.float32)
    for jw in range(NW1):
        for i in range(8):
            eyesel1[i, 32 * jw + 4 * jw + i] = 1.0
    brput("eyesel1", 8, eyesel1)
    eyesel2 = np.zeros((8, 32 * NW2), np.float32)
    for jw in range(NW2):
        for i in range(8):
            eyesel2[i, 32 * jw + 8 * jw + i] = 1.0
    brput("eyesel2", 8, eyesel2)

    br = round_fp32r(br)

    bf = np.zeros((128, BF_COLS), np.float32)

    def bfput(name, rows, arr):
        c0, w = BF_LAY[name]
        a = np.asarray(arr, np.float32)
        assert a.shape == (rows, w), (name, a.shape)
        bf[0:rows, c0:c0 + w] = a

    s1, b1 = fold_bn(inp["bn1"])
    s2, b2 = fold_bn(inp["bn2"])
    s3, b3 = fold_bn(inp["bn3"])
    # pool-before-activation needs positive BN scales
    assert (s1 > 0).all() and (s2 > 0).all() and (s3 > 0).all()
    sa1, ba1 = fold_bn(inp["bnA1"])
    sa2, ba2 = fold_bn(inp["bnA2"])
    sm1f, bm1f = fold_bn(inp["bnM1"], extra_bias=inp["th1_b"], post_scale=0.5)
    sm2f, bm2f = fold_bn(inp["bnM2"], extra_bias=inp["th2_b"], post_scale=0.5)

    cv = np.zeros((128, NCV), np.float32)
    cv[0:64, C_S1] = s1
    cv[64:128, C_S1] = s1
    cv[0:64, C_B1] = b1
    cv[64:128, C_B1] = b1
    cv[:, C_S2], cv[:, C_B2] = s2, b2
    cv[:, C_S3], cv[:, C_B3] = s3, b3
    cv[:, C_SA1_0], cv[:, C_SA1_1] = sa1[0:128], sa1[128:256]
    cv[:, C_BA1_0], cv[:, C_BA1_1] = ba1[0:128], ba1[128:256]
    cv[:, C_SA2_0], cv[:, C_SA2_1] = sa2[0:128], sa2[128:256]
    cv[:, C_BA2_0], cv[:, C_BA2_1] = ba2[0:128], ba2[128:256]
    cv[:, C_SM1], cv[:, C_BM1] = sm1f, bm1f
    cv[:, C_SM2], cv[:, C_BM2] = sm2f, bm2f
    fb1 = np.asarray(inp["fc1_b"], np.float32)
    fb2 = np.asarray(inp["fc2_b"], np.float32)
    cv[:, C_FB1_0], cv[:, C_FB1_1] = fb1[0:128], fb1[128:256]
    cv[:, C_FB2_0], cv[:, C_FB2_1] = fb2[0:128], fb2[128:256]
    cv[:, C_FB3] = np.asarray(inp["fc3_b"], np.float32)
    g1b = np.asarray(inp["g1_b"], np.float32)
    g2b = np.asarray(inp["g2_b"], np.float32)
    cv[:, C_GB1_0], cv[:, C_GB1_1] = g1b[0:128], g1b[128:256]
    cv[:, C_GB2_0], cv[:, C_GB2_1] = g2b[0:128], g2b[128:256]
    cv[0:8, C_FB4] = np.pad(np.asarray(inp["fc4_b"], np.float32), (0, 3))
    bfput("cv", 128, cv)
    bfput("ident", 128, np.eye(128, dtype=np.float32))

    t_of = np.arange(8) // NNODE
    mask8 = DECAY ** np.abs(t_of[:, None] - t_of[None, :]).astype(np.float32)
    eye8 = np.eye(8, dtype=np.float32)
    bfput("mask8", 8, np.tile(mask8, (1, NGT)))
    bfput("fb4t", 8, np.tile(np.pad(np.asarray(inp["fc4_b"], np.float32),
                                    (0, 3))[:, None], (1, BSH)))

    g, be, m, v = (np.asarray(inp["bn_map2"][i], np.float64) for i in range(4))
    sc = r32(g / np.sqrt(v + 1e-5))
    bi = be + (np.asarray(inp["map2_b"], np.float64) - m) * (g / np.sqrt(v + 1e-5))
    # cv scale cols are per-partition = output channel chunks
    cv[:, C_SMAP_0], cv[:, C_SMAP_1] = sc[0:128], sc[128:256]
    bfput("cv", 128, cv)
    pe = pos_encoding().astype(np.float64)
    btot = np.zeros((R, D2), np.float64)
    for rr in range(R):
        btot[rr] = bi + pe[(rr // NNODE) % TLEN]
    btot = r32(btot)
    # transposed+padded bias table: btotT[c_part, 128h + 32b + k]
    btotT = np.zeros((128, D2), np.float32)
    for h in range(2):
        for b in range(BSH):
            for k in range(24):
                btotT[:, 128 * h + 32 * b + k] = btot[24 * b + k,
                                                      128 * h:128 * (h + 1)]
    bfput("btotT", 128, btotT)

    mw = np.asarray(inp["map2_w"], np.float32).reshape(LSTMO, CONV_OUT, D2)
    wprime = bf16(mw.transpose(1, 0, 2).reshape(FEAT_IN, D2))

    return {
        "blob_r": br, "blob_f": bf, "wprime": wprime,
        "fc1w": bf16(inp["fc1_w"]),
        "w2tb": bf16(np.concatenate(
            [w2t, chunks2(inp["g1_w"]), chunks2(inp["g2_w"]),
             chunks2(inp["th1_w"]), chunks2(inp["th2_w"])], axis=1)),
    }


def _mm(nc, out, lhsT, rhs, **kw):
    if lhsT.dtype == F32:
        lhsT = lhsT.bitcast(F32R)
    if rhs.dtype == F32:
        rhs = rhs.bitcast(F32R)
    nc.tensor.matmul(out, lhsT, rhs, **kw)


def build_program():
    nc = bacc.Bacc("TRN2", target_bir_lowering=False, debug=False)

    t1all_d = nc.dram_tensor("t1all", [3 * R, 1502], F32R, kind="ExternalInput")
    br_d = nc.dram_tensor("blob_r", [128, BR_COLS], F32R, kind="ExternalInput")
    bf_d = nc.dram_tensor("blob_f", [128, BF_COLS], F32, kind="ExternalInput")
    wp_d = nc.dram_tensor("wprime", [FEAT_IN, D2], BF16, kind="ExternalInput")
    w2b_d = nc.dram_tensor("w2tb", [128, 1920], BF16, kind="ExternalInput")
    f1_d = nc.dram_tensor("fc1w", [4096, D2], BF16, kind="ExternalInput")
    out_d = nc.dram_tensor("out", [BSH, 5], F32, kind="ExternalOutput")

    with tile.TileContext(nc) as tc, ExitStack() as st:
        persist = st.enter_context(tc.tile_pool(name="persist", bufs=1))

        # conv-critical consts first; everything else queued behind them
        brsb = persist.tile([128, BR_COLS], F32R)
        bfsb = persist.tile([128, BF_COLS], F32)
        nc.sync.dma_start(brsb[:, 0:128], br_d[:, 0:128])
        w2sb = persist.tile([128, 1920], BF16)
        nc.sync.dma_start(w2sb[:, 0:384], w2b_d[:, 0:384])
        nc.sync.dma_start(bfsb[:, 0:BF_CONV], bf_d[:, 0:BF_CONV])
        nc.sync.dma_start(brsb[:, 128:BR_CONV], br_d[:, 128:BR_CONV])

        def brs(name, rows=128):            # fp32r blob slice
            c0, w = BR_LAY[name]
            return brsb[0:rows, c0:c0 + w]

        def bfs(name, rows=128):            # fp32 blob slice
            c0, w = BF_LAY[name]
            return bfsb[0:rows, c0:c0 + w]

        def cvcol(i):
            c0, _ = BF_LAY["cv"]
            return bfsb[:, c0 + i:c0 + i + 1]

        identsb = bfs("ident")

        a3 = persist.tile([128, R * P3], BF16)          # conv3 out, [o, r*191+l]
        a3v = a3.rearrange("p (r l) -> p r l", l=P3)

        # manual rings with one-time pad zeroing (pads are read by tap slides)
        p1a_ring = [persist.tile([128, 771], BF16, name=f"p1a{i}")
                    for i in range(4)]
        s2a_ring = [persist.tile([128, 384], F32R, name=f"s2a{i}")
                    for i in range(5)]
        for t_ in p1a_ring:
            nc.vector.memset(t_[:, 0:2], 0.0)
            nc.vector.memset(t_[:, 753:771], 0.0)
        for t_ in s2a_ring:
            nc.vector.memset(t_[:, 0:3].bitcast(F32), 0.0)
            nc.vector.memset(t_[:, 380:384].bitcast(F32), 0.0)

        # ======== wprime ring (bf16), lives through conv + map2 ========
        wring_cm = tc.tile_pool(name="wring", bufs=N_WRING)
        wring = wring_cm.__enter__()
        wtiles = {}

        def wp_load(wi):
            l0 = W_TILE * wi
            nch = min(W_TILE, P3 - l0)
            wt = wring.tile([128, W_TILE * D2], BF16, tag="wp")
            nc.sync.dma_start(
                wt.rearrange("p (n c) -> p n c", c=D2)[:, 0:nch, :],
                wp_d[128 * l0:128 * (l0 + nch), :]
                .rearrange("(n p) c -> p n c", p=128),
            )
            wtiles[wi] = wt

        # ======== conv stack ========
        # engine split: ACT does the two big BN+ReLUs (e1, e2); Pool engine
        # pools conv1 + conv3 (SBUF-side); DVE pools conv2 and does conv3's
        # scale+bias (relu folds into the pool's max-with-0).
        with (
            tc.tile_pool(name="t1", bufs=2) as t1p,
            tc.tile_pool(name="e1", bufs=3) as e1p,
            tc.tile_pool(name="e2", bufs=2) as e2p,
            tc.tile_pool(name="e3", bufs=3) as e3p,
            tc.tile_pool(name="psc1", bufs=1, space="PSUM") as psc1,
            tc.tile_pool(name="psc2", bufs=2, space="PSUM") as psc2,
            tc.tile_pool(name="psc3", bufs=1, space="PSUM") as psc3,
        ):
            def t1_load(pg):
                t1 = t1p.tile([6, PGRP * 1502], F32R, name=f"t1g")
                nc.gpsimd.dma_start(
                    t1.rearrange("k (pl c) -> k pl c", c=1502)[:],
                    t1all_d[6 * PGRP * pg:6 * PGRP * (pg + 1), :]
                    .rearrange("(pl k) c -> k pl c", k=6),
                )
                return t1

            NGROUPS = PAIRS // PGRP
            t1tiles = {0: t1_load(0)}

            def stage_a(p):
                # conv1 of pair p (+ group-boundary prefetches)
                pg, pl = divmod(p, PGRP)
                if pl == 0:
                    if pg + 1 < NGROUPS:
                        t1tiles[pg + 1] = t1_load(pg + 1)
                    if pg < N_WRING:
                        wp_load(pg)
                    if pg == 1:
                        nc.sync.dma_start(brsb[:, BR_CONV:], br_d[:, BR_CONV:])
                        nc.sync.dma_start(w2sb[:, 384:], w2b_d[:, 384:])
                        nc.sync.dma_start(bfsb[:, BF_CONV:], bf_d[:, BF_CONV:])
                t1 = t1tiles[pg]
                bc = 1502 * pl
                ps1 = psc1.tile([128, L1], F32)
                for c0, c1 in ((0, 512), (512, 1024), (1024, L1)):
                    _mm(nc, ps1[:, c0:c1], brs("b1blk", 6),
                        t1[:, bc + c0:bc + c1])
                e1 = e1p.tile([128, L1], F32)
                nc.scalar.activation(e1[:], ps1[:], ACTF.Relu,
                                     bias=cvcol(C_B1), scale=cvcol(C_S1))
                pa = p1a_ring[p % 4]
                nc.vector.tensor_max(
                    pa[:, 3:752], e1[:, 1:1498:2], e1[:, 2:1499:2])
                nc.vector.tensor_copy(
                    pa[:, 2:753:750], e1[:, 0:L1:L1 - 1])
                if pl == PGRP - 1:
                    t1tiles.pop(pg)

            def stage_b(p):
                # conv2 of pair p's two rows (matmuls first, then epilogues)
                pa = p1a_ring[p % 4]
                ps2s = []
                for ri in range(2):
                    base = 64 * ri
                    ps2 = psc2.tile([128, 753], F32)
                    for t in range(KCONV):
                        lhs = w2sb[base:base + 64, 128 * t:128 * (t + 1)]
                        _mm(nc, ps2[:, 0:512], lhs,
                            pa[base:base + 64, t:t + 512],
                            start=(t == 0), stop=(t == 2))
                        _mm(nc, ps2[:, 512:753], lhs,
                            pa[base:base + 64, t + 512:t + 753],
                            start=(t == 0), stop=(t == 2))
                    ps2s.append(ps2)
                for ri in range(2):
                    r = 2 * p + ri
                    ps2 = ps2s[ri]
                    e2 = e2p.tile([128, L2], F32)
                    sa_ = s2a_ring[r % 5]
                    if r % 6 == 5:
                        # DVE-only path (ACT relief): scale+bias without
                        # relu, then fold relu into the pooling max
                        nc.vector.tensor_scalar(e2[:], ps2[:, 0:L2],
                                                cvcol(C_S2), cvcol(C_B2),
                                                op0=AL.mult, op1=AL.add)
                        nc.vector.scalar_tensor_tensor(
                            sa_[:, 4:380], e2[:, 1:752:2], 0.0,
                            e2[:, 2:753:2], op0=AL.max, op1=AL.max)
                        nc.vector.tensor_scalar_max(sa_[:, 3:4],
                                                    e2[:, 0:1], 0.0)
                    else:
                        nc.scalar.activation(e2[:], ps2[:, 0:L2],
                                             ACTF.Relu, bias=cvcol(C_B2),
                                             scale=cvcol(C_S2))
                        nc.vector.tensor_max(
                            sa_[:, 4:380], e2[:, 1:752:2], e2[:, 2:753:2])
                        nc.vector.tensor_copy(sa_[:, 3:4], e2[:, 0:1])

            def stage_c(r):
                # conv3 + pooling of one row
                sa_ = s2a_ring[r % 5]
                ps3 = psc3.tile([128, L3 + 1], F32)
                for t in range(KCONV):
                    _mm(nc, ps3[:], brs("w3t")[:, 128 * t:128 * (t + 1)],
                        sa_[:, t:t + L3 + 1], start=(t == 0), stop=(t == 2))
                e3 = e3p.tile([128, L3], F32)
                if r % 2 == 0:
                    nc.scalar.activation(e3[:], ps3[:, 0:L3], ACTF.Relu,
                                         bias=cvcol(C_B3), scale=cvcol(C_S3))
                    nc.vector.tensor_max(
                        a3v[:, r, 1:P3], e3[:, 1:380:2], e3[:, 2:381:2])
                    nc.vector.tensor_copy(a3v[:, r, 0:1], e3[:, 0:1])
                else:
                    nc.vector.tensor_scalar(e3[:], ps3[:, 0:L3],
                                            cvcol(C_S3), cvcol(C_B3),
                                            op0=AL.mult, op1=AL.add)
                    nc.vector.scalar_tensor_tensor(
                        a3v[:, r, 1:P3], e3[:, 1:380:2], 0.0,
                        e3[:, 2:381:2], op0=AL.max, op1=AL.max)
                    nc.vector.tensor_scalar_max(a3v[:, r, 0:1],
                                                e3[:, 0:1], 0.0)

            # 4-stage software pipeline: every cross-engine hand-off gets a
            # full tick (~3.5us of PE work) of slack. The last pair's odd
            # row is pulled one tick forward to shorten the drain.
            for i in range(PAIRS + 2):
                if i < PAIRS:
                    stage_a(i)
                if 1 <= i <= PAIRS:
                    stage_b(i - 1)
                if 2 <= i <= PAIRS + 1:
                    stage_c(2 * (i - 2))
                if 3 <= i <= PAIRS:
                    stage_c(2 * (i - 3) + 1)
                if i == PAIRS + 1:
                    stage_c(2 * (PAIRS - 2) + 1)
                    stage_c(2 * (PAIRS - 1) + 1)

        # ======== map2 (bf16 stream) ========
        araw = persist.tile([R, D2], F32)
        with tc.tile_pool(name="mp2", bufs=1, space="PSUM") as mp2:
            psm = mp2.tile([R, D2], F32)
            for wi in range(N_WTILES):
                if wi not in wtiles:
                    wp_load(wi)
                wt = wtiles.pop(wi)
                l0 = W_TILE * wi
                nch = min(W_TILE, P3 - l0)
                for k in range(nch):
                    l = l0 + k
                    _mm(nc, psm[:], a3v[:, :, l], wt[:, D2 * k:D2 * (k + 1)],
                        start=(l == 0), stop=(l == P3 - 1))
            nc.vector.tensor_copy(araw[:], psm[:])
        wring_cm.__exit__(None, None, None)

        # prefetch fc1 chunks now (bf16, 2MB) so the FC head never stalls
        fring_cm = tc.tile_pool(name="fring", bufs=FC_CHUNKS)
        fring = fring_cm.__enter__()
        wcs = []
        for ci in range(FC_CHUNKS):
            wc = fring.tile([128, 4 * D2], BF16, tag="f1w")
            nc.sync.dma_start(
                wc.rearrange("p (n c) -> p n c", c=D2)[:],
                f1_d[512 * ci:512 * (ci + 1), :]
                .rearrange("(n p) c -> p n c", p=128),
            )
            wcs.append(wc)

        # ======== MPNN blocks (batch-padded row layout r' = 32b + r%24) ====
        o1 = persist.tile([128, NNODE * NG1], BF16)   # block out^T [c,(b,jw,n)]
        o2 = persist.tile([128, NNODE * NG2], BF16)
        with (
            tc.tile_pool(name="mp_sb", bufs=2) as msb,
            tc.tile_pool(name="mp_ps", bufs=1, space="PSUM") as mps,
            tc.tile_pool(name="mp_tp", bufs=2, space="PSUM") as mtp,
            tc.tile_pool(name="mp_tpa", bufs=1, space="PSUM") as mtpa,
            tc.tile_pool(name="mp_bp", bufs=1, space="PSUM") as mbp,
        ):
            # aft_pad[h]: [128, 128] fp32r, cols r' = 32b + (r % 24), pads 0
            # BN(map2) applied transposed: per-partition scale + btotT table
            aft = []
            for h in range(2):
                pt = mtp.tile([128, R], F32, tag="tp")
                nc.tensor.transpose(pt[:], araw[:, 128 * h:128 * (h + 1)],
                                    identsb[0:R, 0:R])
                t_ = persist.tile([128, 128], BF16, tag=f"aftp{h}")
                nc.vector.memset(t_[:], 0.0)
                # all 4 batch quads in one op: per-operand strides differ
                # (dst cols 32b+k, src cols 24b+k, table cols 128h+32b+k)
                nc.vector.scalar_tensor_tensor(
                    t_.rearrange("p (b k) -> p b k", k=32)[:, :, 0:24],
                    pt.rearrange("p (b k) -> p b k", k=24)[:],
                    cvcol(C_SMAP_0 + h),
                    bfs("btotT")[:, 128 * h:128 * (h + 1)]
                    .rearrange("p (b k) -> p b k", k=32)[:, :, 0:24],
                    op0=AL.mult, op1=AL.add)
                aft.append(t_)

            BLK = [  # (G, nw, stride, gcols0) per block; blk2 cols follow blk1
                (NG1, NW1, 1, 0),
                (NG2, NW2, 2, NG1),
            ]

            def blkc(blk, lst):
                return lst[blk]

            # --- phase A: graph logits for BOTH blocks into one psum tile
            awall = mps.tile([8, 8 * NGT], F32, tag="awall")
            # -1e8*eye pre-fill of every graph's logits in one matmul
            _mm(nc, awall[:], brs("eyen8", 8), brs("identt", 8),
                start=True, stop=False)
            for blk in range(2):
                G, nw, stride, g0 = BLK[blk]
                gw = blkc(blk, (w2sb[:, 384:896], w2sb[:, 896:1408]))
                gbc = blkc(blk, (C_GB1_0, C_GB2_0))
                ft = []
                for h in range(2):
                    ftp = mps.tile([128, 128], F32, tag="ftp")
                    for dh in range(2):
                        _mm(nc, ftp[:],
                            gw[:, 256 * dh + 128 * h:256 * dh + 128 * h + 128],
                            aft[dh][:], start=(dh == 0), stop=(dh == 1))
                    t_ = msb.tile([128, 128], F32R, tag=f"ft{h}")
                    nc.scalar.activation(t_[:], ftp[:], ACTF.Identity,
                                         bias=cvcol(gbc + h), scale=1.0)
                    ft.append(t_)
                for b in range(BSH):
                    for jw in range(nw):
                        g = g0 + nw * b + jw
                        rb = 32 * b + 4 * stride * jw
                        for h in range(2):
                            _mm(nc, awall[:, 8 * g:8 * (g + 1)],
                                ft[h][:, rb:rb + 8], ft[h][:, rb:rb + 8],
                                start=False, stop=(h == 1))

            # --- one softmax/mask chain over all 32 graphs
            # Adj = softmax(lrelu(L - 1e8 eye)) * mask + eye   (rows = i)
            NGA = 8 * NGT
            aw2 = msb.tile([8, NGA], F32, tag="aw2")
            aw3 = msb.tile([8, NGA], F32, tag="aw3")
            aw4 = msb.tile([8, NGA], F32, tag="aw4")
            aw5 = msb.tile([8, NGA], F32, tag="aw5")
            aw6 = msb.tile([8, NGA], F32, tag="aw6")
            rmax = msb.tile([8, NGT], F32, tag="rmax")
            rsum = msb.tile([8, NGT], F32, tag="rsum")
            rrec = msb.tile([8, NGT], F32, tag="rrec")
            nc.scalar.activation(aw2[:], awall[:], ACTF.Lrelu, alpha=0.01)
            a3d = aw2.rearrange("p (g j) -> p g j", j=8)
            nc.vector.reduce_max(rmax[:], a3d[:], axis=mybir.AxisListType.X)
            nc.vector.tensor_sub(aw3.rearrange("p (g j) -> p g j", j=8)[:],
                                 a3d[:],
                                 rmax[:].unsqueeze(2).broadcast_to([8, NGT, 8]))
            nc.scalar.activation(aw4[:], aw3[:], ACTF.Exp)
            nc.vector.reduce_sum(rsum[:],
                                 aw4.rearrange("p (g j) -> p g j", j=8)[:],
                                 axis=mybir.AxisListType.X)
            nc.vector.reciprocal(rrec[:], rsum[:])
            nc.vector.tensor_mul(aw5.rearrange("p (g j) -> p g j", j=8)[:],
                                 aw4.rearrange("p (g j) -> p g j", j=8)[:],
                                 rrec[:].unsqueeze(2).broadcast_to([8, NGT, 8]))
            nc.vector.tensor_mul(aw6[:], aw5[:], bfs("mask8", 8))

            # --- phase B: adjacency scatter + message passing, both blocks
            pshall = mps.tile([128, 8 * NGT], F32, tag="pshall")
            for blk in range(2):
                G, nw, stride, g0 = BLK[blk]
                thw = blkc(blk, (w2sb[:, 1408:1664], w2sb[:, 1664:1920]))
                sel = blkc(blk, (brs("sel1", 40), brs("sel2", 24)))
                eyesel = blkc(blk, (brs("eyesel1", 8), brs("eyesel2", 8)))
                nsel = blkc(blk, (40, 24))
                sa = blkc(blk, ((C_SA1_0, C_SA1_1), (C_SA2_0, C_SA2_1)))
                ba = blkc(blk, ((C_BA1_0, C_BA1_1), (C_BA2_0, C_BA2_1)))

                # Adj^T per batch quad: adjt_all[8jw+j, 8b+i] = Adj_g[i, j]
                pta = mtpa.tile([nsel, 32], F32, tag="tpa")
                for b in range(BSH):
                    nc.tensor.transpose(
                        pta[:, 8 * b:8 * (b + 1)],
                        aw6[:, 8 * (g0 + nw * b):8 * (g0 + nw * (b + 1))],
                        identsb[0:8, 0:8])
                adjt = msb.tile([nsel, 32], F32R, tag="adjt")
                nc.vector.tensor_copy(adjt[:], pta[:])

                # banded B quads (partitions 0:32, col-chunked per batch):
                #   B_b[k, 8jw+i] = sum_m Sel_jw[m, k] adjt[m, 8b+i]
                bps = mbp.tile([32, 8 * nw * BSH], F32, tag="bps")
                for b in range(BSH):
                    for jw in range(nw):
                        c0 = 8 * nw * b + 8 * jw
                        _mm(nc, bps[0:32, c0:c0 + 8],
                            eyesel[:, 32 * jw:32 * (jw + 1)],
                            brs("ident8", 8), start=True, stop=False)
                        _mm(nc, bps[0:32, c0:c0 + 8],
                            sel[:, 32 * jw:32 * (jw + 1)],
                            adjt[:, 8 * b:8 * (b + 1)],
                            start=False, stop=True)
                bsb = msb.tile([32, 8 * nw * BSH], F32R, tag="bsb")
                nc.vector.tensor_copy(bsb[:], bps[:])

                # Xb^T = BN_A(AF^T); Y_pad = Xb @ thw  [128 r', 128]
                xbt = []
                for h in range(2):
                    t_ = msb.tile([128, 128], BF16, tag=f"xbt{h}")
                    nc.scalar.activation(t_[:], aft[h][:], ACTF.Identity,
                                         bias=cvcol(ba[h]), scale=cvcol(sa[h]))
                    xbt.append(t_)
                psy = mps.tile([128, HID], F32, tag="psy")
                for h in range(2):
                    _mm(nc, psy[:], xbt[h][:], thw[:, HID * h:HID * (h + 1)],
                        start=(h == 0), stop=(h == 1))
                # Y quads down to partitions 0:32 (col-chunked per batch)
                ysbq = msb.tile([32, HID * BSH], F32R, tag="ysbq")
                for b in range(BSH):
                    nc.vector.tensor_copy(ysbq[0:32, HID * b:HID * (b + 1)],
                                          psy[32 * b:32 * (b + 1), :])

                # h^T: psh[c, 8(g0 + nw b + jw) + i] via 4 K=32 matmuls
                for b in range(BSH):
                    c0 = 8 * (g0 + nw * b)
                    _mm(nc, pshall[:, c0:c0 + 8 * nw],
                        ysbq[0:32, HID * b:HID * (b + 1)],
                        bsb[0:32, 8 * nw * b:8 * nw * (b + 1)])

            # lrelu(psh * sM + bM)  (incl th_b, BN_M, x0.5 mean-fold)
            hp = msb.tile([128, 8 * NGT], F32, tag="hp")
            nc.scalar.activation(hp[:, 0:8 * NG1], pshall[:, 0:8 * NG1],
                                 ACTF.Lrelu, alpha=0.01,
                                 bias=cvcol(C_BM1), scale=cvcol(C_SM1))
            nc.scalar.activation(hp[:, 8 * NG1:], pshall[:, 8 * NG1:],
                                 ACTF.Lrelu, alpha=0.01,
                                 bias=cvcol(C_BM2), scale=cvcol(C_SM2))
            hpv = hp.rearrange("p (g j) -> p g j", j=8)
            nc.vector.tensor_add(o1.rearrange("p (g n) -> p g n", n=4)[:],
                                 hpv[:, 0:NG1, 0:4], hpv[:, 0:NG1, 4:8])
            nc.vector.tensor_add(o2.rearrange("p (g n) -> p g n", n=4)[:],
                                 hpv[:, NG1:NGT, 0:4], hpv[:, NG1:NGT, 4:8])

        # ======== FC head (transposed throughout) ========
        with (
            tc.tile_pool(name="fc_sb", bufs=2) as fsb_p,
            tc.tile_pool(name="fc_ps", bufs=1, space="PSUM") as fps,
        ):
            o1v = o1.rearrange("p (b k) -> p b k", k=4 * NW1)
            o2v = o2.rearrange("p (b k) -> p b k", k=4 * NW2)
            pst = [fps.tile([128, BSH], F32, name=f"pst1_{h}", tag=f"p1_{h}")
                   for h in range(2)]
            ci = 0
            npairs = (NW1 + NW2) * NNODE             # 32 accumulation steps
            for njw, o_v, ci0 in ((NW1, o1v, 0), (NW2, o2v, NW1)):
                for jw in range(njw):
                    wc = wcs[ci0 + jw]
                    wcv = wc.rearrange("p (n c) -> p n c", c=D2)
                    for n in range(NNODE):
                        for h in range(2):
                            _mm(nc, pst[h][:],
                                wcv[:, n, 128 * h:128 * (h + 1)],
                                o_v[:, :, 4 * jw + n],
                                start=(ci == 0), stop=(ci == npairs - 1))
                        ci += 1

            def relu_t(ps_list, bias_cols):
                outs = []
                for h, ps_ in enumerate(ps_list):
                    t_ = fsb_p.tile([128, BSH], F32R, tag=f"h{bias_cols[h]}")
                    nc.scalar.activation(t_[:], ps_[:], ACTF.Relu,
                                         bias=cvcol(bias_cols[h]), scale=1.0)
                    outs.append(t_)
                return outs

            h1t = relu_t(pst, (C_FB1_0, C_FB1_1))
            pst2 = [fps.tile([128, BSH], F32, name=f"pst2_{h}", tag=f"p2_{h}")
                    for h in range(2)]
            for ch in range(2):
                for dh in range(2):
                    _mm(nc, pst2[ch][:],
                        brs("fc2w")[:, 256 * dh + 128 * ch:256 * dh + 128 * ch + 128],
                        h1t[dh][:], start=(dh == 0), stop=(dh == 1))
            h2t = relu_t(pst2, (C_FB2_0, C_FB2_1))
            pst3 = fps.tile([128, BSH], F32, tag="p3")
            for dh in range(2):
                _mm(nc, pst3[:], brs("fc3w")[:, 128 * dh:128 * dh + 128],
                    h2t[dh][:], start=(dh == 0), stop=(dh == 1))
            h3t = relu_t([pst3], (C_FB3,))
            pst4 = fps.tile([8, BSH], F32, tag="p4")
            _mm(nc, pst4[:], brs("fc4w"), h3t[0][:])
            osb = fsb_p.tile([8, BSH], F32, tag="osb")
            nc.scalar.activation(osb[:], pst4[:], ACTF.Identity,
                                 bias=cvcol(C_FB4)[0:8, :])
            # transpose on the way out: out[b, j] = osb[j, b]
            dst = dataclasses.replace(out_d[:], ap=[[1, 5], [5, 4]], offset=0)
            nc.sync.dma_start(dst, osb[0:5, :])

        fring_cm.__exit__(None, None, None)

    nc.compile()
    return nc


_CACHE = {}


def _get_program():
    if "nc" not in _CACHE:
        _CACHE["nc"] = build_program()
    return _CACHE["nc"]


def make_in_maps(inputs):
    consts = prep_consts(inputs)
    x = np.asarray(inputs["X"], np.float32).reshape(BS * TLEN * NNODE, DIM)
    in_maps = []
    for c in range(NCORES):
        shard = x[R * c:R * (c + 1)]
        xp = np.zeros((R, 1504), np.float32)
        xp[:, 1:1 + DIM] = shard
        # t1all[3r + t, c] = xpad[r, c + t]
        sw = np.lib.stride_tricks.sliding_window_view(xp, 1502, axis=1)[:, 0:3]
        t1all = round_fp32r(sw.reshape(3 * R, 1502))
        m = {"t1all": t1all}
        m.update(consts)
        in_maps.append(m)
    return in_maps


def kernel(**inputs):
    from concourse.bass_utils import run_bass_kernel_spmd

    nc = _get_program()
    in_maps = make_in_maps(inputs)
    res = run_bass_kernel_spmd(nc, in_maps, core_ids=list(range(NCORES)))
    outs = [np.asarray(res.results[c]["out"]) for c in range(NCORES)]
    return np.concatenate(outs, axis=0).astype(np.float32)
